# revision 30
# baseline (speedup 1.0000x reference)
"""Trainium2 Bass kernel for nn_BertSelfOutputPAL (v4).

Data-parallel over batch: 8 batch elements -> 8 NeuronCores, no collectives.
Per core (batch element b), with S=2048, H=1024, P=256, T=4:
  h   = hs @ Wd (+db)                    (dense)
  low_t = h @ W1[t] (+b1[t])             (PAL down-proj, T branches)
  ts_t  = low_t @ W2[t] (+b2[t])         (PAL up-proj)
  tw  = softmax(h @ encW + mask)         (token gate over S)
  tv  = tw @ h ; td = softmax(tv @ selW.T + selb)
  x   = h + input + sum_t td[t] * ts_t ; out = LayerNorm(x)*g + beta

Structure (v4 changes vs v3 marked *):
  - hs is transposed on the host and uploaded feature-major in bf16; dense
    runs in bf16.  PAL branches run fp8e4m3 DoubleRow (weights host-scaled
    by 64, h evicted to fp8 at 8x); hT stored at 4096x, undone at LN.
  - * input_tensor is uploaded feature-major fp32 (at 4096x) and added into
    hT by accum-DMA (cce add) per chunk during phase 1, right after the
    token-gate pass reads hT.  The phase-3 back-transposes then carry
    h+db+input for free: no per-tile input DMA and no fp32r inject matmuls
    in phase 3, and the 8MB output stream owns phase-3 DMA bandwidth.
  - * token-gate logits come from h8 via fp8 DoubleRow (encW fp8 at 512x,
    exp scale folds 1/4096), replacing 1-row bf16 matmuls on X.
  - * td logits accumulate into a dedicated PSUM bank chunk-by-chunk
    (matmul tvp[:,kt,c] x selW), removing the serial tvs reduction from
    the phase-2 critical path.
  - td gates the PAL combine via in-place td scaling of the fp8 W2 tiles;
    phase-3 psum groups are [4 hT-transposes, (b2), 4 stage2-DR], so the
    transposes (no td/W2 dependency) run ahead while W2 is scaled.
  - * persist-pool init (identity etc.) is deferred until after the
    priority DMAs are issued; chunk-0 X/Wd loads are split per-kt so the
    first dense matmul starts as early as possible; bulk loads (W1/W2,
    later X chunks) issue at staggered points so they don't steal head
    bandwidth.
  - hardware gotchas honored: DVE must not read bf16, psum accumulation
    groups must not interleave within a bank.
"""

import numpy as np
import ml_dtypes
from contextlib import ExitStack

import concourse.bacc as bacc
import concourse.mybir as mybir
import concourse.tile as tile
from concourse.bass_utils import run_bass_kernel_spmd
from concourse.masks import make_identity

FP = mybir.dt.float32
FR = mybir.dt.float32r
BF = mybir.dt.bfloat16
F8 = mybir.dt.float8e4
AF = mybir.ActivationFunctionType
ALU = mybir.AluOpType
AX = mybir.AxisListType
PM = mybir.MatmulPerfMode
EPS = 1e-12

B, S_FULL, H, P, T = 8, 2048, 1024, 256, 4
KT = H // 128       # 8 h-tiles
PT = P // 128       # 2 p-tiles
N_CORES = 8

H8S = 8.0           # h -> fp8 scale
WS = 64.0           # W1, W2 host scale
ES = 512.0          # encW -> fp8 scale
IDS = 4096.0        # PAL psum scale = (8*64/512)*64*64 ; hT stored at IDS
EV = 1.0 / IDS
LSC = 1.0 / (H8S * ES)   # logits psum -> logits

F8NP = ml_dtypes.float8_e4m3
BFNP = ml_dtypes.bfloat16


def fr(ap):
    return ap.bitcast(FR)


def build_nc(S=S_FULL, zb2=False, zmask=False, zg=False, zb=False):
    SC = S // 512            # 512-wide s-chunks
    nc = bacc.Bacc("TRN2", target_bir_lowering=False, debug=False)

    # ---- DRAM I/O (per-core) ----
    xt_d = nc.dram_tensor("xt", [128, SC, KT, 512], BF, kind="ExternalInput").ap()
    inpt_d = nc.dram_tensor("inpt", [SC, 2, 128, KT // 2, 512], FP,
                            kind="ExternalInput").ap()
    mask_d = nc.dram_tensor("mask", [1, S], FP, kind="ExternalInput").ap()
    wd_d = nc.dram_tensor("wd", [128, KT, H], BF, kind="ExternalInput").ap()
    dbias_d = nc.dram_tensor("dbias", [128, KT], FP, kind="ExternalInput").ap()
    encw8_d = nc.dram_tensor("encw8", [128, KT, 128], F8,
                             kind="ExternalInput").ap()
    vw_d = nc.dram_tensor("vw", [128, KT, T], FR, kind="ExternalInput").ap()
    selb_d = nc.dram_tensor("selb", [1, T], FP, kind="ExternalInput").ap()
    w1_d = nc.dram_tensor("w1", [T, 128, KT, P], F8, kind="ExternalInput").ap()
    w2_d = nc.dram_tensor("w2", [T, 128, PT, H], F8, kind="ExternalInput").ap()
    b1_d = nc.dram_tensor("b1", [128, PT, T], FP, kind="ExternalInput").ap()
    b2_d = nc.dram_tensor("b2", [T, H], FR, kind="ExternalInput").ap()
    lng_d = nc.dram_tensor("lng", [1, H], FP, kind="ExternalInput").ap()
    lnb_d = nc.dram_tensor("lnb", [1, H], FP, kind="ExternalInput").ap()
    outp = nc.dram_tensor("out", [S, H], FP, kind="ExternalOutput").ap()

    with tile.TileContext(nc) as tc, ExitStack() as ctx:
        # ---------- persistent pools ----------
        persist = ctx.enter_context(tc.tile_pool(name="persist", bufs=1))
        htp = ctx.enter_context(tc.tile_pool(name="htp", bufs=1))

        ident = persist.tile([128, 128], FP, tag="ident", name="ident")
        identr = persist.tile([128, 128], FR, tag="identr", name="identr")
        ones1f = persist.tile([1, 128], FP, tag="ones1f", name="ones1f")
        ones1 = persist.tile([1, 128], FR, tag="ones1", name="ones1")
        epst = persist.tile([128, 1], FP, tag="epst", name="epst")
        zerot = persist.tile([128, 1], FP, tag="zerot", name="zerot")

        dbias = persist.tile([128, KT], FP, tag="dbias", name="dbias")
        dbias4k = persist.tile([128, KT], FP, tag="dbias4k", name="dbias4k")
        dbias8 = persist.tile([128, KT], FP, tag="dbias8", name="dbias8")
        # encW fp8, replicated across 128 stationary columns so the DR
        # logits matmul has a full-width (ISA-valid) lhsT; every psum row
        # then holds the logits, giving the partition-broadcast for free
        encw8 = persist.tile([128, KT, 128], F8, tag="encw8", name="encw8")
        vw_sb = persist.tile([128, KT, T], FR, tag="vw_sb", name="vw_sb")
        selb = persist.tile([1, T], FP, tag="selb", name="selb")
        b1s = persist.tile([128, PT, T], FP, tag="b1s", name="b1s")
        lngb = None if zg else persist.tile([128, H], FP, tag="lngb", name="lngb")
        lnbb = None if zb else persist.tile([128, H], FP, tag="lnbb", name="lnbb")

        # online-softmax / gating state (zsc row 0 is the real Z partials)
        zsc = persist.tile([128, SC], FP, tag="zsc", name="zsc")
        tvp = persist.tile([128, KT, SC], FR, tag="tvp", name="tvp")
        td_row = persist.tile([1, T], FP, tag="td_row", name="td_row")
        tdcol = persist.tile([T, 1], FP, tag="tdcol", name="tdcol")
        b2c = persist.tile([1, H], FP, tag="b2c", name="b2c")
        tdb = persist.tile([128, T], FP, tag="tdb", name="tdb")

        # hT: feature-major (h+db) fp32 at IDS scale; later += IDS*input
        hT = [htp.tile([128, S], FP, tag=f"ht{k}", name=f"ht{k}") for k in range(KT)]
        # h8: feature-major h fp8 (x8), DoubleRow-sliceable [128, KT, S]
        h8 = htp.tile([128, KT, S], F8, tag="h8", name="h8")

        # PAL weights + low tiles (SBUF lifetime spans phases 1-3)
        w12 = ctx.enter_context(tc.tile_pool(name="w12", bufs=1))
        W1sb, W2sb = [], []
        low8p = ctx.enter_context(tc.tile_pool(name="low8", bufs=SC))
        low_tiles = {}

        # ================= phase 1: dense + online logits/tv + low ===========
        with tc.tile_pool(name="pA", bufs=1) as pa, \
             tc.tile_pool(name="pA_xt", bufs=3) as xtp, \
             tc.tile_pool(name="pA_inp", bufs=4) as inpp, \
             tc.tile_pool(name="pA_twb", bufs=2) as twbp, \
             tc.tile_pool(name="pA_scr", bufs=1) as scrp, \
             tc.tile_pool(name="lowps", bufs=2, space="PSUM") as lowps, \
             tc.tile_pool(name="tdpp", bufs=1, space="PSUM") as tdpp, \
             tc.tile_pool(name="pA_ps_d", bufs=1, space="PSUM") as dps, \
             tc.tile_pool(name="pA_ps_l", bufs=1, space="PSUM") as lps:

            # ---- priority DMAs: chunk-0 XT / Wd interleaved per-kt so the
            # first dense matmul's operands land first; small params go on
            # the scalar queue; bulk loads are issued later at staggered
            # points.
            XTc = [None] * SC
            XTc[0] = xtp.tile([128, KT, 512], BF, tag="xtc", name="xtc0")
            Wd_sb = [pa.tile([128, H], BF, tag=f"wd{k}", name=f"wd{k}")
                     for k in range(KT)]
            for kt in range(KT):
                nc.sync.dma_start(XTc[0][:, kt:kt + 1, :], xt_d[:, 0, kt:kt + 1, :])
                nc.sync.dma_start(Wd_sb[kt][:, 0:512], wd_d[:, kt, 0:512])
            nc.scalar.dma_start(dbias[:], dbias_d)
            nc.scalar.dma_start(encw8[:], encw8_d)
            for kt in range(KT):
                nc.sync.dma_start(Wd_sb[kt][:, 512:1024], wd_d[:, kt, 512:1024])
            nc.scalar.dma_start(vw_sb[:], vw_d)
            nc.scalar.dma_start(selb[:], selb_d)
            nc.scalar.dma_start(b1s[:], b1_d)
            if not zmask:
                mrow = pa.tile([1, S], FP, tag="mrow", name="mrow")
                nc.scalar.dma_start(mrow[:], mask_d)
            if not zb2:
                b2n = pa.tile([T, H], FR, tag="b2n", name="b2n")
                nc.scalar.dma_start(b2n[:], b2_d)

            # ---- deferred preamble init (off the head critical path) ----
            make_identity(nc, ident[:])
            nc.scalar.copy(identr[:], ident[:])
            nc.gpsimd.memset(ones1f[:], 1.0)
            nc.scalar.copy(ones1[:], ones1f[:])
            nc.gpsimd.memset(epst[:], EPS)
            nc.gpsimd.memset(zerot[:], 0.0)
            nc.vector.tensor_scalar(dbias4k[:], dbias[:], IDS, None, op0=ALU.mult)
            nc.vector.tensor_scalar(dbias8[:], dbias[:], H8S, None, op0=ALU.mult)
            if not zg:
                lngr = pa.tile([1, H], FP, tag="lngr", name="lngr")
                nc.scalar.dma_start(lngr[:], lng_d)
                nc.gpsimd.partition_broadcast(lngb[:], lngr[:])
            if not zb:
                lnbr = pa.tile([1, H], FP, tag="lnbr", name="lnbr")
                nc.scalar.dma_start(lnbr[:], lnb_d)
                nc.gpsimd.partition_broadcast(lnbb[:], lnbr[:])

            tdps = tdpp.tile([1, T], FP, tag="tdps", name="tdps")

            def evict_h(mt, c0, width, ps_ap):
                # both evictions on Act (phase-1 DVE is the tighter engine:
                # it carries the token-gate scr ops and the low evictions)
                nc.scalar.activation(
                    fr(hT[mt][:, c0:c0 + width]), ps_ap, AF.Identity,
                    bias=dbias4k[:, mt:mt + 1], scale=IDS)
                nc.scalar.activation(
                    h8[:, mt, c0:c0 + width], ps_ap, AF.Identity,
                    bias=dbias8[:, mt:mt + 1], scale=H8S)

            def tdps_partial(c):
                # td logits accumulate chunk-by-chunk: psum += tvp[:,kt,c]^T@vw
                for kt in range(KT):
                    nc.tensor.matmul(
                        tdps[:], tvp[:, kt, c:c + 1], vw_sb[:, kt, :],
                        start=(c == 0 and kt == 0),
                        stop=(c == SC - 1 and kt == KT - 1))

            def do_logits(c):
                if c >= 1:
                    tdps_partial(c - 1)
                # logits = encW8^T @ h8 (fp8 DoubleRow, encW replicated over
                # the 128 stationary columns -> every psum row = logits);
                # db.encW term constant under softmax; 1/(8*512) scale folds
                # into the exp.  Logits are data-bounded (|l| < ~10) and
                # masks <= 0, so exp() cannot overflow: no max-subtraction.
                c0 = c * 512
                lpsum = lps.tile([128, 512], FP, tag="lps", name="lps")
                for g in range(KT // 2):
                    nc.tensor.matmul(
                        lpsum[:], encw8[:, 2 * g:2 * g + 2, :],
                        h8[:, 2 * g:2 * g + 2, c0:c0 + 512],
                        start=(g == 0), stop=(g == KT // 2 - 1),
                        perf_mode=PM.DoubleRow)
                twb = twbp.tile([128, 512], FP, tag="twb", name="twb")
                if zmask:
                    nc.scalar.activation(twb[:], lpsum[:], AF.Exp,
                                         bias=0.0, scale=LSC,
                                         accum_out=zsc[:, c:c + 1])
                else:
                    mrowb = twbp.tile([128, 512], FP, tag="mrb", name="mrb")
                    nc.gpsimd.partition_broadcast(mrowb[:],
                                                  mrow[:, c0:c0 + 512])
                    nc.vector.scalar_tensor_tensor(
                        twb[:], lpsum[:], LSC, mrowb[:],
                        op0=ALU.mult, op1=ALU.add)
                    nc.scalar.activation(twb[:], twb[:], AF.Exp,
                                         bias=0.0, scale=1.0,
                                         accum_out=zsc[:, c:c + 1])
                scr = scrp.tile([128, 512], FP, tag="scr", name="scr")
                for kt in range(KT):
                    nc.vector.scalar_tensor_tensor(
                        scr[:], hT[kt][:, c0:c0 + 512], 1.0, twb[:],
                        op0=ALU.mult, op1=ALU.mult,
                        accum_out=tvp[:, kt, c:c + 1])

            inp_tiles = {}

            def inp_load(c):
                # stage IDS*input (feature-major) for chunk c
                for half in range(2):
                    it = inpp.tile([128, KT // 2, 512], FP, tag="inpt",
                                   name="inpt")
                    nc.sync.dma_start(it[:], inpt_d[c, half])
                    inp_tiles[(c, half)] = it

            def inp_add(c):
                # hT += IDS*input on GpSimd (idle engine), after the
                # token-gate read of hT; fr() write keeps the fp32r
                # consumers (phase-3 transposes) happy
                c0 = c * 512
                for mt in range(KT):
                    it = inp_tiles.pop((c, mt // 4)) if mt % 4 == 3 else \
                        inp_tiles[(c, mt // 4)]
                    nc.gpsimd.tensor_add(fr(hT[mt][:, c0:c0 + 512]),
                                         hT[mt][:, c0:c0 + 512],
                                         it[:, mt % 4, :])

            def emit_low_t(c, t):
                # PAL down-proj for chunk c, task t: fp8 DoubleRow; no td
                # dependency (td is applied later via in-place W2 scaling).
                lt = low8p.tile([128, PT, 512], F8, tag=f"low{t}",
                                name=f"low{t}")
                low_tiles[(c, t)] = lt
                for pt in range(PT):
                    ps = lowps.tile([128, 512], FP, tag="lowps", name="lowps")
                    for g in range(KT // 2):
                        nc.tensor.matmul(
                            ps[:],
                            W1sb[t][:, 2 * g:2 * g + 2,
                                    pt * 128:(pt + 1) * 128],
                            h8[:, 2 * g:2 * g + 2, c * 512:(c + 1) * 512],
                            start=(g == 0), stop=(g == KT // 2 - 1),
                            perf_mode=PM.DoubleRow,
                        )
                    # low8 = psum/8 + 64*b1  [= 64*(low+b1)]
                    nc.vector.tensor_scalar(
                        lt[:, pt, :], ps[:], 1.0 / H8S,
                        b1s[:, pt:pt + 1, t:t + 1],
                        op0=ALU.mult, op1=ALU.add)

            # --- chunk 0: kt-outer in two mt-group passes so the PE can start
            # as soon as the first kt slice of XT chunk 0 + Wd lands
            for grp in range(2):
                pss = [dps.tile([128, 512], FP, tag=f"dd{m}",
                                name=f"c0_{grp}_{m}") for m in range(4)]
                for kt in range(KT):
                    for m in range(4):
                        mt = grp * 4 + m
                        nc.tensor.matmul(
                            pss[m][:],
                            Wd_sb[kt][:, mt * 128:(mt + 1) * 128],
                            XTc[0][:, kt, :],
                            start=(kt == 0), stop=(kt == KT - 1),
                        )
                if grp == 0:
                    # bulk loads, staggered: XT chunk 1 + PAL W1 after the
                    # first dense group's matmuls are issued
                    XTc[1] = xtp.tile([128, KT, 512], BF, tag="xtc",
                                      name="xtc1")
                    nc.sync.dma_start(XTc[1][:], xt_d[:, 1, :, :])
                    for t in range(T):
                        w1t = w12.tile([128, KT, P], F8, tag=f"w1_{t}",
                                       name=f"w1_{t}")
                        nc.sync.dma_start(w1t[:], w1_d[t])
                        W1sb.append(w1t)
                    inp_load(0)
                else:
                    for t in range(T):
                        w2t = w12.tile([128, PT, H], F8, tag=f"w2_{t}",
                                       name=f"w2_{t}")
                        nc.sync.dma_start(w2t[:], w2_d[t])
                        W2sb.append(w2t)
                    if SC > 2:
                        XTc[2] = xtp.tile([128, KT, 512], BF, tag="xtc",
                                          name="xtc2")
                        nc.sync.dma_start(XTc[2][:], xt_d[:, 2, :, :])
                for m in range(4):
                    evict_h(grp * 4 + m, 0, 512, pss[m][:])
            do_logits(0)
            inp_add(0)

            # --- chunks 1..SC-1: mt-outer dense, with prev chunk's low
            # groups interleaved between dense mt-groups so a psum stall in
            # one stream lets the other proceed
            for c in range(1, SC):
                if c + 2 < SC:
                    XTc[c + 2] = xtp.tile([128, KT, 512], BF, tag="xtc",
                                          name=f"xtc{c + 2}")
                    nc.sync.dma_start(XTc[c + 2][:], xt_d[:, c + 2, :, :])
                inp_load(c)
                for mt in range(KT):
                    ps = dps.tile([128, 512], FP, tag=f"dd{mt % 4}",
                                  name=f"dd{mt}")
                    for kt in range(KT):
                        nc.tensor.matmul(
                            ps[:],
                            Wd_sb[kt][:, mt * 128:(mt + 1) * 128],
                            XTc[c][:, kt, :],
                            start=(kt == 0), stop=(kt == KT - 1),
                        )
                    evict_h(mt, c * 512, 512, ps[:])
                    if mt % 2 == 1:
                        emit_low_t(c - 1, mt // 2)
                do_logits(c)
                inp_add(c)
            for t in range(T):
                emit_low_t(SC - 1, t)
            tdps_partial(SC - 1)

            # ---------- phase 2: softmax normalizer, td, scale W2 ----------
            pb = pa
            Zt = pb.tile([1, 1], FP, tag="Zt", name="Zt")
            nc.vector.reduce_sum(Zt[:], zsc[0:1, :], axis=AX.X)
            rZ = pb.tile([1, 1], FP, tag="rZ", name="rZ")
            nc.vector.reciprocal(rZ[:], Zt[:])
            rZE = pb.tile([1, 1], FP, tag="rZE", name="rZE")
            nc.vector.tensor_scalar(rZE[:], rZ[:], EV, None, op0=ALU.mult)
            tdl = pb.tile([1, T], FP, tag="tdl", name="tdl")
            nc.vector.scalar_tensor_tensor(tdl[:], tdps[:], rZE[:], selb[:],
                                           op0=ALU.mult, op1=ALU.add)
            z2 = pb.tile([1, 1], FP, tag="z2", name="z2")
            nc.scalar.activation(tdl[:], tdl[:], AF.Exp, bias=0.0,
                                 scale=1.0, accum_out=z2[:])
            rz2 = pb.tile([1, 1], FP, tag="rz2", name="rz2")
            nc.vector.reciprocal(rz2[:], z2[:])
            nc.vector.tensor_scalar(fr(td_row[:]), tdl[:], rz2[:], None,
                                    op0=ALU.mult)
            nc.gpsimd.partition_broadcast(tdb[:], td_row[:])
            # scale W2 in place by td (Act engine; fp8 in/out), half-H at a
            # time so stage2 can start after the first half
            for hc in range(2):
                for t in range(T):
                    nc.scalar.activation(
                        W2sb[t][:, :, hc * 512:(hc + 1) * 512],
                        W2sb[t][:, :, hc * 512:(hc + 1) * 512],
                        AF.Identity, bias=0.0, scale=tdb[:, t:t + 1])
            if not zb2:
                ps2 = lps.tile([T, 2], FP, tag="lps", name="tdc")
                nc.tensor.matmul(ps2[:], fr(td_row[:]), ones1[:, :2],
                                 start=True, stop=True)
                nc.scalar.copy(fr(tdcol[:]), ps2[:, :1])
                for hc in range(2):
                    ps3 = lps.tile([1, 512], FP, tag="lps", name="b2ps")
                    nc.tensor.matmul(ps3[:], fr(tdcol[:]),
                                     b2n[:, hc * 512:(hc + 1) * 512],
                                     start=True, stop=True)
                    # b2c at stage2 psum scale (x IDS)
                    nc.vector.tensor_scalar(fr(b2c[:, hc * 512:(hc + 1) * 512]),
                                            ps3[:], IDS, None, op0=ALU.mult)

        # ================= phase 3: transposes + stage2 (fp8 DR) + LN ========
        xps = ctx.enter_context(tc.tile_pool(name="xps", bufs=8, space="PSUM"))
        xt_pool = ctx.enter_context(tc.tile_pool(name="xt3", bufs=3))
        stats = ctx.enter_context(tc.tile_pool(name="stats", bufs=4))
        scrq = xt_pool.tile([128, H], FP, tag="scrq", name="scrq")

        for c in range(SC):
            for st in range(4):
                s_abs = c * 4 + st
                pss = []
                for hc in range(2):
                    # group order: hT back-transposes first (no W2/td
                    # dependency, keeps the PE busy over the phase-2
                    # boundary); hT carries h+db+input
                    # start=True on the first write marks the whole bank
                    # pending-zero; the other transposes overwrite their
                    # (still-pending) 128-col regions rather than accumulate
                    ps = xps.tile([128, 512], FP, tag="xps", name="xps")
                    for j in range(4):
                        kt = hc * 4 + j
                        nc.tensor.matmul(
                            fr(ps[:, j * 128:(j + 1) * 128]),
                            fr(hT[kt][:, s_abs * 128:(s_abs + 1) * 128]),
                            identr[:],
                            is_transpose=True, start=(j == 0), stop=False,
                        )
                    if not zb2:
                        nc.tensor.matmul(
                            ps[:], ones1[:], fr(b2c[:, hc * 512:(hc + 1) * 512]),
                            start=False, stop=False,
                        )
                    for t in range(T):
                        nc.tensor.matmul(
                            ps[:],
                            low_tiles[(c, t)][:, :, st * 128:(st + 1) * 128],
                            W2sb[t][:, :, hc * 512:(hc + 1) * 512],
                            start=False, stop=(t == T - 1),
                            perf_mode=PM.DoubleRow,
                        )
                    pss.append(ps)
                # ---- x = psum/IDS; LayerNorm spread over Act/DVE/GpSimd so
                # no single engine gates the phase-3 pipeline.  Both evicts
                # on DVE (its accumulator read is ~free; Act's costs 183ns);
                # Act carries square+sqrt+norm-half; GpSimd the tiny stats
                # via varH = ssq + negmu*ssum (Pool runs tensor_tensor only).
                xt_ = xt_pool.tile([128, H], FP, tag="x", name="x")
                s0 = stats.tile([128, 1], FP, tag="s0", name="s0")
                s1 = stats.tile([128, 1], FP, tag="s1", name="s1")
                nc.vector.tensor_scalar(xt_[:, :512], pss[0][:], EV, 0.0,
                                        op0=ALU.mult, op1=ALU.add,
                                        accum_out=s0[:])
                nc.vector.tensor_scalar(xt_[:, 512:], pss[1][:], EV, 0.0,
                                        op0=ALU.mult, op1=ALU.add,
                                        accum_out=s1[:])
                ssq = stats.tile([128, 1], FP, tag="ssq", name="ssq")
                nc.scalar.activation(scrq[:], xt_[:], AF.Square, bias=zerot[:],
                                     accum_out=ssq[:])
                ssum = stats.tile([128, 1], FP, tag="ssum", name="ssum")
                nc.gpsimd.tensor_add(ssum[:], s0[:], s1[:])
                negmu = stats.tile([128, 1], FP, tag="negmu", name="negmu")
                nc.vector.tensor_scalar(negmu[:], ssum[:], -1.0 / H, None,
                                        op0=ALU.mult)
                nmus = stats.tile([128, 1], FP, tag="nmus", name="nmus")
                nc.gpsimd.tensor_mul(nmus[:], negmu[:], ssum[:])
                varh = stats.tile([128, 1], FP, tag="varh", name="varh")
                nc.gpsimd.tensor_add(varh[:], ssq[:], nmus[:])
                sd = stats.tile([128, 1], FP, tag="sd", name="sd")
                nc.scalar.activation(sd[:], varh[:], AF.Sqrt, bias=epst[:],
                                     scale=1.0 / H)
                isd = stats.tile([128, 1], FP, tag="isd", name="isd")
                nc.vector.reciprocal(isd[:], sd[:])
                nmi = stats.tile([128, 1], FP, tag="nmi", name="nmi")
                nc.gpsimd.tensor_mul(nmi[:], negmu[:], isd[:])
                # norm halves: Act computes isd*x + (-mu*isd); DVE (x-mu)*isd
                nc.scalar.activation(xt_[:, :512], xt_[:, :512], AF.Identity,
                                     bias=nmi[:], scale=isd[:])
                nc.vector.tensor_scalar(xt_[:, 512:], xt_[:, 512:], negmu[:],
                                        isd[:], op0=ALU.add, op1=ALU.mult)
                if not zg:
                    nc.vector.scalar_tensor_tensor(xt_[:], xt_[:], 1.0, lngb[:],
                                                   op0=ALU.mult, op1=ALU.mult)
                if not zb:
                    nc.gpsimd.tensor_add(xt_[:], xt_[:], lnbb[:])
                nc.sync.dma_start(outp[s_abs * 128:(s_abs + 1) * 128, :], xt_[:])

    nc.finalize()
    return nc


_CACHE = {}


def _get_nc(S=S_FULL, zb2=False, zmask=False, zg=False, zb=False):
    key = (S, zb2, zmask, zg, zb)
    if key not in _CACHE:
        _CACHE[key] = build_nc(S, zb2=zb2, zmask=zmask, zg=zg, zb=zb)
    return _CACHE[key]


def _flags(inputs):
    f32 = lambda x: np.asarray(x, dtype=np.float32)
    return dict(
        zb2=not np.any(f32(inputs["pal_b2"])),
        zmask=not np.any(f32(inputs["attention_mask"])),
        zg=bool(np.all(f32(inputs["ln_g"]) == 1.0)),
        zb=not np.any(f32(inputs["ln_b"])),
    )


def _in_maps(inputs, S=S_FULL):
    SC = S // 512
    f32 = lambda x: np.ascontiguousarray(np.asarray(x), dtype=np.float32)
    hs = f32(inputs["hidden_states"])
    inp = f32(inputs["input_tensor"]) * IDS
    msk = f32(inputs["attention_mask"]).reshape(B, S)
    Wd = f32(inputs["dense_W"])
    db = f32(inputs["dense_b"])
    encw = f32(inputs["enc_W"])
    selw = f32(inputs["sel_W"])  # [T, H]
    encw8 = np.ascontiguousarray(np.broadcast_to(
        np.clip(encw * ES, -448, 448).reshape(KT, 128).T[:, :, None],
        (128, KT, 128))).astype(F8NP)
    vw = selw.T.reshape(KT, 128, T).transpose(1, 0, 2).copy()  # [128,KT,T]
    selb_eff = f32(inputs["sel_b"]).reshape(1, T)
    dbias = db.reshape(KT, 128).T.copy()
    wd_dev = Wd.reshape(KT, 128, H).transpose(1, 0, 2).copy().astype(BFNP)
    W1 = f32(inputs["pal_W1"]) * WS
    w1_dev = W1.reshape(T, KT, 128, P).transpose(0, 2, 1, 3).copy().astype(F8NP)
    W2 = f32(inputs["pal_W2"]) * WS
    w2_dev = W2.reshape(T, PT, 128, H).transpose(0, 2, 1, 3).copy().astype(F8NP)
    b1 = f32(inputs["pal_b1"]).reshape(T, PT, 128).transpose(2, 1, 0).copy() * WS
    b2 = f32(inputs["pal_b2"])
    lng = f32(inputs["ln_g"]).reshape(1, H)
    lnb = f32(inputs["ln_b"]).reshape(1, H)
    shared = dict(wd=wd_dev, dbias=dbias, encw8=encw8, vw=vw, selb=selb_eff,
                  w1=w1_dev, w2=w2_dev, b1=b1, b2=b2, lng=lng, lnb=lnb)
    out = []
    for bi in range(B):
        xt = hs[bi].reshape(SC, 512, KT, 128).transpose(3, 0, 2, 1).copy()
        # feature-major IDS*input, [SC, half, 128, kt4, 512]
        inpt = np.ascontiguousarray(
            inp[bi].T.reshape(2, KT // 2, 128, SC, 512).transpose(3, 0, 2, 1, 4))
        out.append(dict(xt=xt.astype(BFNP), inpt=inpt,
                        mask=msk[bi:bi + 1], **shared))
    return out


def kernel(**inputs):
    nc = _get_nc(**_flags(inputs))
    res = run_bass_kernel_spmd(nc, _in_maps(inputs), list(range(N_CORES)))
    out = np.stack([res.results[b]["out"] for b in range(B)], axis=0)
    return out
